# revision 1
# baseline (speedup 1.0000x reference)
"""Self-contained TRN2 Bass kernel for the RGCN message-passing problem.

kernel(**inputs) takes the FULL unsharded inputs (text, src, dst, rel,
bases, comp, bias), shards edges by destination window across the 8
NeuronCores, runs the SPMD Bass program via run_bass_kernel_spmd, and
returns the full [64, 512, 256] float32 output.
"""

import numpy as np
import ml_dtypes

import concourse.bass as bass
import concourse.tile as tile
from concourse import bacc, mybir

F = 256      # in features
O = 256      # out features
NB = 3       # bases
WINDOW = 64  # dst rows per window
GBUFS = 6    # gather tile buffering depth (feeds 4 SWDGE queues)
NQ = 4       # SWDGE queues


def plan_calls(slot_cws, cpc):
    """Split each window slot into gather calls of <= cpc chunks.
    Returns list of (slot, chunk_lo, n_chunks) in execution order."""
    calls = []
    for i, cw in enumerate(slot_cws):
        lo = 0
        while lo < cw:
            n = min(cpc, cw - lo)
            calls.append((i, lo, n))
            lo += n
    return calls


def build_program(n_nodes, slot_cws, cpc=8, n_cores=8):
    slot_cws = list(slot_cws)
    nw = len(slot_cws)
    nchunks = sum(slot_cws)
    epad = nchunks * 128
    dcore = nw * WINDOW
    calls = plan_calls(slot_cws, cpc)
    ncalls = len(calls)

    bf16 = mybir.dt.bfloat16
    f32 = mybir.dt.float32
    i16 = mybir.dt.int16
    i32 = mybir.dt.int32

    # bf16 DRAM I/O breaks NEFF load under the PJRT path; all bf16 payloads
    # travel as int16 containers and are bitcast on-chip.
    nc = bacc.Bacc("TRN2", target_bir_lowering=False, debug=False,
                   num_devices=n_cores, num_swdge_queues=NQ)
    h_d = nc.dram_tensor("h", [n_nodes, F], i16, kind="ExternalInput").ap()
    gidx_d = nc.dram_tensor("gidx", [128, epad // 16], i16,
                            kind="ExternalInput").ap()
    w1h_d = nc.dram_tensor("w1h", [128, nchunks, NB * WINDOW], i16,
                           kind="ExternalInput").ap()
    cnt_d = nc.dram_tensor("cnt", [1, ncalls], i32, kind="ExternalInput").ap()
    bases_d = nc.dram_tensor("bases", [NB, F, O], i16,
                             kind="ExternalInput").ap()
    bias_d = nc.dram_tensor("bias", [1, O], i16, kind="ExternalInput").ap()
    out_d = nc.dram_tensor("out", [dcore, O], i16, kind="ExternalOutput").ap()

    with tile.TileContext(nc) as tc:
        with (
            tc.tile_pool(name="const", bufs=1) as cpool,
            tc.tile_pool(name="gather", bufs=GBUFS) as gpool,
            tc.tile_pool(name="w1h", bufs=4) as wpool,
            tc.tile_pool(name="abt", bufs=2) as apool,
            tc.tile_pool(name="ost", bufs=2) as opool,
            tc.tile_pool(name="ps1", bufs=2, space="PSUM") as ps1,
            tc.tile_pool(name="ps2", bufs=2, space="PSUM") as ps2,
        ):
            # ---- prologue ----
            gidx_sb = cpool.tile([128, epad // 16], i16)
            # call 0's slice first so the gather pipeline starts immediately
            ntot = epad // 16
            cuts = [0, calls[0][2] * 8]
            cuts += [cuts[1] + (ntot - cuts[1]) * k // 3 for k in (1, 2, 3)]
            for lo, hi in zip(cuts[:-1], cuts[1:]):
                if hi > lo:
                    nc.sync.dma_start(gidx_sb[:, lo:hi], gidx_d[:, lo:hi])
            bases_i = cpool.tile([128, NB, 2, O], i16)
            for b in range(NB):
                for h in range(2):
                    nc.sync.dma_start(bases_i[:, b, h, :],
                                      bases_d[b, h * 128:(h + 1) * 128, :])
            bias_i = cpool.tile([1, O], i16)
            nc.sync.dma_start(bias_i[:], bias_d[:])
            bias_sb = bias_i[:].bitcast(bf16)
            ones_sb = cpool.tile([1, WINDOW], bf16)
            nc.vector.memset(ones_sb[:], 1.0)

            # ---- main pipeline ----
            p1 = [None, None]
            chunk_base = 0
            for j, (slot, clo, ncall) in enumerate(calls):
                nidx = ncall * 128
                G = gpool.tile([128, cpc, F], i16, tag="G", name="G")
                # pads carry index 0 (a real row; W1h zeroes their weights),
                # so every gathered row is valid and no trim/memset is needed
                nc.gpsimd.dma_gather(
                    G[:, 0:ncall, :], h_d[:],
                    gidx_sb[:, chunk_base * 8:(chunk_base + ncall) * 8],
                    nidx, nidx, F, queue_num=j % NQ)
                W = wpool.tile([128, cpc, NB * WINDOW], i16, tag="W", name="W")
                nc.sync.dma_start(
                    W[:, 0:ncall, :],
                    w1h_d[:, chunk_base:chunk_base + ncall, :])
                for c in range(ncall):
                    cw_pos = clo + c
                    if cw_pos == 0:
                        p1 = [ps1.tile([128, NB * WINDOW], f32,
                                       tag=f"p1h{h}", name=f"p1h{h}")
                              for h in range(2)]
                    last = (cw_pos == slot_cws[slot] - 1)
                    for h in range(2):
                        nc.tensor.matmul(
                            p1[h][:],
                            G[:, c, h * 128:(h + 1) * 128].bitcast(bf16),
                            W[:, c, :].bitcast(bf16),
                            start=(cw_pos == 0), stop=last)
                    if last:
                        abt = apool.tile([128, 2, NB * WINDOW], bf16,
                                         tag="abt", name="abt")
                        for h in range(2):
                            nc.vector.tensor_copy(abt[:, h, :], p1[h][:])
                        p2 = ps2.tile([WINDOW, O], f32, tag="p2", name="p2")
                        nc.tensor.matmul(p2[:], ones_sb[:], bias_sb,
                                         start=True, stop=False)
                        for b in range(NB):
                            for h in range(2):
                                nc.tensor.matmul(
                                    p2[:],
                                    abt[:, h, b * WINDOW:(b + 1) * WINDOW],
                                    bases_i[:, b, h, :].bitcast(bf16),
                                    start=False,
                                    stop=(b == NB - 1 and h == 1))
                        osb = opool.tile([WINDOW, O], bf16, tag="osb",
                                         name="osb")
                        nc.scalar.activation(
                            osb[:], p2[:], mybir.ActivationFunctionType.Relu)
                        nc.sync.dma_start(
                            out_d[slot * WINDOW:(slot + 1) * WINDOW, :],
                            osb[:].bitcast(i16))
                chunk_base += ncall

    nc.compile()
    return nc


def host_prep(src, dst, rel, comp, n_nodes, n_cores, cpc=8):
    """Sort/deal/pad edges; build the streamed W1h relayout."""
    dcore = n_nodes // n_cores
    nw = dcore // WINDOW
    ngw = n_cores * nw
    w_edge = comp[rel].astype(ml_dtypes.bfloat16)        # [E, NB]
    gw = (dst // WINDOW).astype(np.int64)
    order = np.argsort(gw, kind="stable")
    counts = np.bincount(gw, minlength=ngw)
    starts = np.concatenate([[0], np.cumsum(counts)])

    # deal windows to cores by descending count; slot capacity = group max
    ranked = np.argsort(-counts, kind="stable")
    slot_cws = [max(1, -(-int(counts[ranked[n_cores * i]]) // 128))
                for i in range(nw)]
    calls = plan_calls(slot_cws, cpc)
    nchunks = sum(slot_cws)
    epad = nchunks * 128
    ncalls = len(calls)

    gidx = np.zeros((n_cores, epad), np.int16)
    w1h = np.zeros((n_cores, epad, NB * WINDOW), ml_dtypes.bfloat16)
    cnts = np.zeros((n_cores, 1, ncalls), np.int32)
    win_of_slot = np.zeros((n_cores, nw), np.int64)
    dstloc = (dst % WINDOW).astype(np.int64)

    slot_base = np.zeros(nw, np.int64)
    acc = 0
    for i, cw in enumerate(slot_cws):
        slot_base[i] = acc
        acc += cw
    bidx = np.arange(NB) * WINDOW
    for k in range(n_cores):
        for i in range(nw):
            wid = int(ranked[n_cores * i + k])
            win_of_slot[k, i] = wid
            es = order[starts[wid]:starts[wid + 1]]
            base = slot_base[i] * 128
            n = len(es)
            gidx[k, base:base + n] = src[es].astype(np.int16)
            pos = base + np.arange(n)
            w1h[k, pos[:, None], bidx[None, :] + dstloc[es][:, None]] = \
                w_edge[es]
    for j, (slot, clo, ncall) in enumerate(calls):
        base = (slot_base[slot] + clo) * 128
        seg = gidx[:, base:base + ncall * 128]
        cnts[:, 0, j] = (seg >= 0).sum(axis=1)

    # wrap gidx: idx i -> partition i%16, slot i//16; replicate to 128 parts
    gidx_w = gidx.reshape(n_cores, epad // 16, 16).transpose(0, 2, 1)
    gidx_w = np.tile(gidx_w, (1, 8, 1)).copy()
    # w1h layout: edge e -> [e%128, e//128, :]
    w1h_t = w1h.reshape(n_cores, nchunks, 128, NB * WINDOW)
    w1h_t = w1h_t.transpose(0, 2, 1, 3).copy()
    return gidx_w, w1h_t, cnts, tuple(slot_cws), win_of_slot


def rgcn_kernel(text, src, dst, rel, bases, comp, bias, n_cores=8,
                run_fn=None, cpc=8, nc_cache={}):
    """Full-input kernel: shard, run on 8 cores, reassemble output."""
    Bt, St, INF = text.shape
    n_nodes = Bt * St
    h = text.reshape(n_nodes, INF)

    src = np.asarray(src).astype(np.int64)
    dst = np.asarray(dst).astype(np.int64)
    rel = np.asarray(rel).astype(np.int64)
    bases_np = np.asarray(bases, np.float32)
    comp_np = np.asarray(comp, np.float32)
    bias_np = np.asarray(bias, np.float32)

    gidx_w, w1h_t, cnts, slot_cws, win_of_slot = host_prep(
        src, dst, rel, comp_np, n_nodes, n_cores, cpc)
    key = (n_nodes, slot_cws, cpc, n_cores)
    if key not in nc_cache:
        nc_cache[key] = build_program(n_nodes, slot_cws, cpc, n_cores)
    nc = nc_cache[key]

    h_bf = np.asarray(h, np.float32).astype(ml_dtypes.bfloat16).view(np.int16)
    bases_bf = bases_np.astype(ml_dtypes.bfloat16).view(np.int16)
    bias_bf = bias_np.reshape(1, O).astype(ml_dtypes.bfloat16).view(np.int16)

    in_maps = [
        dict(h=h_bf, gidx=gidx_w[k], w1h=w1h_t[k].view(np.int16),
             cnt=cnts[k], bases=bases_bf, bias=bias_bf)
        for k in range(n_cores)
    ]
    from concourse.bass_utils import run_bass_kernel_spmd
    if run_fn is None:
        res = run_bass_kernel_spmd(nc, in_maps, list(range(n_cores)))
        outs = [res.results[k]["out"] for k in range(n_cores)]
    else:
        outs = run_fn(nc, in_maps)

    out = np.zeros((n_nodes, O), np.float32)
    nw = len(slot_cws)
    W = WINDOW
    for k in range(n_cores):
        ok = outs[k].view(ml_dtypes.bfloat16).astype(np.float32)
        for i in range(nw):
            wid = win_of_slot[k][i]
            out[wid * W:(wid + 1) * W] = ok[i * W:(i + 1) * W]
    return out.reshape(Bt, St, O)


_NC_CACHE = {}


def kernel(text, src, dst, rel, bases, comp, bias):
    out = rgcn_kernel(
        np.asarray(text, np.float32),
        np.asarray(src), np.asarray(dst), np.asarray(rel),
        np.asarray(bases, np.float32), np.asarray(comp, np.float32),
        np.asarray(bias, np.float32),
        n_cores=8, nc_cache=_NC_CACHE)
    return np.ascontiguousarray(out, np.float32)



# revision 6
# speedup vs baseline: 1.5367x; 1.5367x over previous
"""Self-contained TRN2 Bass kernel for the RGCN message-passing problem.

kernel(**inputs) takes the FULL unsharded inputs (text, src, dst, rel,
bases, comp, bias), shards edges by destination window across the 8
NeuronCores, runs the SPMD Bass program via run_bass_kernel_spmd, and
returns the full [64, 512, 256] float32 output.

Design (v2):
  - Edges are grouped by destination window (W=64 dst rows) and dealt
    to cores by descending window edge-count.  Since the edge indices
    are known on the host, the h[src] gather is done host-side: the
    device streams a pre-gathered [128, nchunks, 256] bf16 tensor with
    large sequential HWDGE DMAs (no SWDGE descriptor generation).
  - The per-edge scatter weights (comp[rel] placed at column
    b*W + dst%W) are built ON-CHIP from an 8-byte/edge metadata stream
    with two DVE ops per slab: onehot = is_equal(iota, dstloc) and
    w1h = onehot * compvals (broadcast APs).
  - Stage 1: per 128-edge chunk, 2 matmuls (G feature-half stationary,
    w1h moving) accumulate A[f, (w2,b,d)] for a PAIR of windows in one
    PSUM tile.
  - Stage 2: flipped — bases halves are the stationary operand (full
    128-wide PE), aggregated features stream as 256 dst columns per
    group of 4 windows.  Bias+ReLU fused in the scalar-engine
    activation; output written as [2, 128, dcore] (feature-major) and
    transposed back on the host.
"""

import numpy as np
import ml_dtypes

import concourse.bass as bass
import concourse.tile as tile
from concourse import bacc, mybir

F = 256       # in features
O = 256       # out features
NB = 3        # bases
W = 64        # dst rows per window
TSLAB = 16    # chunks per DMA slab (16 * 128 * 512B = 1 MiB)
GRP = 4       # windows per stage-2 group (GRP*W = 256 dst cols)


def build_program(n_nodes, slot_cws, n_cores=8):
    slot_cws = list(slot_cws)
    nw = len(slot_cws)
    assert nw % GRP == 0
    nchunks = sum(slot_cws)
    dcore = nw * W
    nslabs = -(-nchunks // TSLAB)
    npairs = nw // 2

    bf16 = mybir.dt.bfloat16
    f32 = mybir.dt.float32
    i16 = mybir.dt.int16

    # bf16 DRAM I/O breaks NEFF load under the PJRT path; all bf16 payloads
    # travel as int16 containers and are bitcast on-chip.
    nc = bacc.Bacc("TRN2", target_bir_lowering=False, debug=False,
                   num_devices=n_cores)
    gh_d = nc.dram_tensor("gh", [128, nchunks, F], i16,
                          kind="ExternalInput").ap()
    meta_d = nc.dram_tensor("meta", [128, nchunks, 4], i16,
                            kind="ExternalInput").ap()
    basesw_d = nc.dram_tensor("basesw", [128, NB, 2, 2, 128], i16,
                              kind="ExternalInput").ap()
    bias_d = nc.dram_tensor("bias", [128, 2], f32, kind="ExternalInput").ap()
    out_d = nc.dram_tensor("out", [2, 128, dcore], i16,
                           kind="ExternalOutput").ap()

    # chunk index -> (slot, is_first_chunk_of_slot, is_last_chunk_of_slot)
    chunk_slot = []
    for s, cw in enumerate(slot_cws):
        for c in range(cw):
            chunk_slot.append((s, c == 0, c == cw - 1))

    with tile.TileContext(nc) as tc:
        with (
            tc.tile_pool(name="const", bufs=1) as cpool,
            tc.tile_pool(name="gh", bufs=3) as gpool,
            tc.tile_pool(name="w1h", bufs=2) as wpool,
            tc.tile_pool(name="oh", bufs=2) as ohpool,
            tc.tile_pool(name="stg", bufs=2) as apool,
            tc.tile_pool(name="ost", bufs=4) as opool,
            tc.tile_pool(name="ps1", bufs=2, space="PSUM") as ps1,
            tc.tile_pool(name="ps2", bufs=2, space="PSUM") as ps2,
        ):
            # ---- prologue: constants ----
            meta_sb = cpool.tile([128, nchunks, 4], i16)
            nc.sync.dma_start(meta_sb[:], meta_d[:])
            meta_bf = meta_sb[:].bitcast(bf16)
            basesw_sb = cpool.tile([128, NB, 2, 2, 128], i16)
            nc.sync.dma_start(basesw_sb[:], basesw_d[:])
            bias_sb = cpool.tile([128, 2], f32)
            nc.sync.dma_start(bias_sb[:], bias_d[:])
            iota_sb = cpool.tile([128, W], bf16)
            nc.gpsimd.iota(iota_sb[:], pattern=[[1, W]], base=0,
                           channel_multiplier=0,
                           allow_small_or_imprecise_dtypes=True)

            slab_tiles = {}

            def ensure_slab(si):
                """Issue the G-slab DMA + on-chip w1h build for slab si."""
                if si in slab_tiles or si >= nslabs:
                    return
                lo = si * TSLAB
                hi = min(nchunks, lo + TSLAB)
                t = hi - lo
                G = gpool.tile([128, TSLAB, F], i16, tag="G", name="G")
                nc.sync.dma_start(G[:, 0:t, :], gh_d[:, lo:hi, :])
                oh = ohpool.tile([128, TSLAB, W], bf16, tag="oh", name="oh")
                dstl = meta_bf[:, lo:hi, 0:1].broadcast_to([128, t, W])
                iota_b = iota_sb[:].unsqueeze(1).broadcast_to([128, t, W])
                nc.vector.tensor_tensor(oh[:, 0:t, :], iota_b, dstl,
                                        mybir.AluOpType.is_equal)
                w1h = wpool.tile([128, TSLAB, NB, W], bf16, tag="W", name="W")
                oh_b = oh[:, 0:t, :].unsqueeze(2).broadcast_to(
                    [128, t, NB, W])
                cv_b = meta_bf[:, lo:hi, 1:4].unsqueeze(3).broadcast_to(
                    [128, t, NB, W])
                nc.vector.tensor_tensor(w1h[:, 0:t, :, :], oh_b, cv_b,
                                        mybir.AluOpType.mult)
                slab_tiles[si] = (G, w1h)

            def emit_stage2(g):
                """Matmul group g's staged features against the bases."""
                stg = staging[g]
                for ohalf in range(2):
                    p2 = ps2.tile([128, GRP * W], f32, tag=f"p2o{ohalf}",
                                  name=f"p2o{ohalf}")
                    k = 0
                    for b in range(NB):
                        for h in range(2):
                            nc.tensor.matmul(
                                p2[:],
                                basesw_sb[:, b, h, ohalf, :].bitcast(bf16),
                                stg[:, h, b, :, :],
                                start=(k == 0), stop=(k == 2 * NB - 1))
                            k += 1
                    osb = opool.tile([128, GRP * W], bf16, tag=f"osb{ohalf}",
                                     name=f"osb{ohalf}")
                    nc.scalar.activation(
                        osb[:], p2[:], mybir.ActivationFunctionType.Relu,
                        bias=bias_sb[:, ohalf:ohalf + 1])
                    nc.sync.dma_start(
                        out_d[ohalf, :, g * GRP * W:(g + 1) * GRP * W],
                        osb[:].bitcast(i16))
                del staging[g]

            # ---- main pipeline over window pairs ----
            staging = {}
            cglob = 0
            ensure_slab(0)
            for j in range(npairs):
                p1 = [ps1.tile([128, 2, NB, W], f32, tag=f"p1h{h}",
                               name=f"p1h{h}") for h in range(2)]
                for w01 in range(2):
                    s = 2 * j + w01
                    for c in range(slot_cws[s]):
                        si, cloc = divmod(cglob, TSLAB)
                        ensure_slab(si)
                        ensure_slab(si + 1)
                        G, w1h = slab_tiles[si]
                        first = (c == 0)
                        last = (c == slot_cws[s] - 1)
                        for h in range(2):
                            nc.tensor.matmul(
                                p1[h][:, w01, :, :],
                                G[:, cloc, h * 128:(h + 1) * 128].bitcast(
                                    bf16),
                                w1h[:, cloc, :, :],
                                start=first, stop=last)
                        cglob += 1
                # defer stage2 of group (j-2)//2 to keep PE fed
                if j >= 2 and j % 2 == 0:
                    emit_stage2((j - 2) // 2)
                g = j // 2
                if g not in staging:
                    staging[g] = apool.tile([128, 2, NB, GRP, W], bf16,
                                            tag="stg", name="stg")
                for h in range(2):
                    dst_ap = staging[g][:, h, :, 2 * (j % 2):2 * (j % 2) + 2,
                                        :].transpose([0, 2, 1, 3])
                    nc.vector.tensor_copy(dst_ap, p1[h][:])
            for g in sorted(staging):
                emit_stage2(g)

    nc.compile()
    return nc


def host_prep(h_bf, src, dst, rel, comp, n_nodes, n_cores):
    """Sort/deal/pad edges; pre-gather h and build the metadata stream."""
    dcore = n_nodes // n_cores
    nw = dcore // W
    ngw = n_cores * nw
    gw = (dst // W).astype(np.int64)
    order = np.argsort(gw, kind="stable")
    counts = np.bincount(gw, minlength=ngw)
    starts = np.concatenate([[0], np.cumsum(counts)])

    # deal windows to cores by descending count; slot capacity = group max
    ranked = np.argsort(-counts, kind="stable")
    slot_cws = [max(1, -(-int(counts[ranked[n_cores * i]]) // 128))
                for i in range(nw)]
    nchunks = sum(slot_cws)
    epad = nchunks * 128

    w_edge = comp[rel].astype(ml_dtypes.bfloat16)        # [E, NB]
    dstloc = (dst % W).astype(np.float32).astype(ml_dtypes.bfloat16)

    gh = np.zeros((n_cores, 128, nchunks, F), np.int16)
    meta = np.zeros((n_cores, 128, nchunks, 4), ml_dtypes.bfloat16)
    win_of_slot = np.zeros((n_cores, nw), np.int64)

    slot_base = np.concatenate([[0], np.cumsum(slot_cws)])[:-1]
    srcs_flat = np.zeros((n_cores, epad), np.int64)
    meta_flat = np.zeros((n_cores, epad, 4), ml_dtypes.bfloat16)
    valid = np.zeros((n_cores, epad), bool)
    for k in range(n_cores):
        for i in range(nw):
            wid = int(ranked[n_cores * i + k])
            win_of_slot[k, i] = wid
            es = order[starts[wid]:starts[wid + 1]]
            base = slot_base[i] * 128
            n = len(es)
            srcs_flat[k, base:base + n] = src[es]
            valid[k, base:base + n] = True
            meta_flat[k, base:base + n, 0] = dstloc[es]
            meta_flat[k, base:base + n, 1:4] = w_edge[es]
    for k in range(n_cores):
        g = h_bf[srcs_flat[k]]                           # [epad, F] int16
        g[~valid[k]] = 0
        gh[k] = g.reshape(nchunks, 128, F).transpose(1, 0, 2)
        meta[k] = meta_flat[k].reshape(nchunks, 128, 4).transpose(1, 0, 2)
    return gh, meta, tuple(slot_cws), win_of_slot


def rgcn_kernel(text, src, dst, rel, bases, comp, bias, n_cores=8,
                run_fn=None, nc_cache={}):
    """Full-input kernel: shard, run on 8 cores, reassemble output."""
    Bt, St, INF = text.shape
    n_nodes = Bt * St
    h = text.reshape(n_nodes, INF)

    src = np.asarray(src).astype(np.int64)
    dst = np.asarray(dst).astype(np.int64)
    rel = np.asarray(rel).astype(np.int64)
    bases_np = np.asarray(bases, np.float32)
    comp_np = np.asarray(comp, np.float32)
    bias_np = np.asarray(bias, np.float32)

    h_bf = np.asarray(h, np.float32).astype(ml_dtypes.bfloat16).view(np.int16)
    gh, meta, slot_cws, win_of_slot = host_prep(
        h_bf, src, dst, rel, comp_np, n_nodes, n_cores)
    key = (n_nodes, slot_cws, n_cores)
    if key not in nc_cache:
        nc_cache[key] = build_program(n_nodes, slot_cws, n_cores)
    nc = nc_cache[key]

    # bases[b, f, o] -> basesw[p, b, h, oh, q] with f = h*128+p, o = oh*128+q
    bw = bases_np.astype(ml_dtypes.bfloat16).view(np.int16)
    basesw = np.ascontiguousarray(
        bw.reshape(NB, 2, 128, 2, 128).transpose(2, 0, 1, 3, 4))
    bias_w = np.ascontiguousarray(
        bias_np.reshape(2, 128).T.astype(np.float32))

    in_maps = [
        dict(gh=gh[k], meta=meta[k].view(np.int16), basesw=basesw,
             bias=bias_w)
        for k in range(n_cores)
    ]
    from concourse.bass_utils import run_bass_kernel_spmd
    if run_fn is None:
        res = run_bass_kernel_spmd(nc, in_maps, list(range(n_cores)))
        outs = [res.results[k]["out"] for k in range(n_cores)]
    else:
        outs = run_fn(nc, in_maps)

    out = np.zeros((n_nodes, O), np.float32)
    nw = len(slot_cws)
    for k in range(n_cores):
        ok = outs[k].view(ml_dtypes.bfloat16).astype(np.float32)
        ok = ok.reshape(O, nw, W)                        # [o, slot, d]
        for i in range(nw):
            wid = win_of_slot[k][i]
            out[wid * W:(wid + 1) * W] = ok[:, i, :].T
    return out.reshape(Bt, St, O)


_NC_CACHE = {}


def kernel(text, src, dst, rel, bases, comp, bias):
    out = rgcn_kernel(
        np.asarray(text, np.float32),
        np.asarray(src), np.asarray(dst), np.asarray(rel),
        np.asarray(bases, np.float32), np.asarray(comp, np.float32),
        np.asarray(bias, np.float32),
        n_cores=8, nc_cache=_NC_CACHE)
    return np.ascontiguousarray(out, np.float32)


# revision 13
# speedup vs baseline: 1.9264x; 1.2536x over previous
"""Self-contained TRN2 Bass kernel for the RGCN message-passing problem.

kernel(**inputs) takes the FULL unsharded inputs (text, src, dst, rel,
bases, comp, bias), shards edges by destination window across the 8
NeuronCores, runs the SPMD Bass program via run_bass_kernel_spmd, and
returns the full [64, 512, 256] float32 output.

Design (v2):
  - Edges are grouped by destination window (W=64 dst rows) and dealt
    to cores by descending window edge-count.  Since the edge indices
    are known on the host, the h[src] gather is done host-side: the
    device streams a pre-gathered [128, nchunks, 256] bf16 tensor with
    large sequential HWDGE DMAs (no SWDGE descriptor generation).
  - The per-edge scatter weights (comp[rel] placed at column
    b*W + dst%W) are built ON-CHIP from an 8-byte/edge metadata stream
    with two DVE ops per slab: onehot = is_equal(iota, dstloc) and
    w1h = onehot * compvals (broadcast APs).
  - Stage 1: per 128-edge chunk, 2 matmuls (G feature-half stationary,
    w1h moving) accumulate A[f, (w2,b,d)] for a PAIR of windows in one
    PSUM tile.
  - Stage 2: flipped — bases halves are the stationary operand (full
    128-wide PE), aggregated features stream as 256 dst columns per
    group of 4 windows.  Bias+ReLU fused in the scalar-engine
    activation; output written as [2, 128, dcore] (feature-major) and
    transposed back on the host.
"""

import numpy as np
import ml_dtypes

import concourse.bass as bass
import concourse.tile as tile
from concourse import bacc, mybir

F = 256       # in features
O = 256       # out features
NB = 3        # bases
W = 32        # dst rows per window
TSLAB = 16    # chunks per DMA slab (16 * 128 * 512B = 1 MiB)
GRP = 256 // W            # windows per stage-2 group (GRP*W = 256 dst cols)
PW = 384 // (NB * W)      # windows per stage-1 PSUM tile (384 f32 cols)


def build_program(n_nodes, slot_cws, n_cores=8):
    slot_cws = list(slot_cws)
    nw = len(slot_cws)
    assert nw % GRP == 0
    nchunks = sum(slot_cws)
    dcore = nw * W
    nslabs = -(-nchunks // TSLAB)
    npairs = nw // PW

    bf16 = mybir.dt.bfloat16
    f32 = mybir.dt.float32
    i16 = mybir.dt.int16

    # bf16 DRAM I/O breaks NEFF load under the PJRT path; all bf16 payloads
    # travel as int16 containers and are bitcast on-chip.
    nc = bacc.Bacc("TRN2", target_bir_lowering=False, debug=False,
                   num_devices=n_cores)
    gh_d = nc.dram_tensor("gh", [128, nchunks, F], i16,
                          kind="ExternalInput").ap()
    meta_d = nc.dram_tensor("meta", [128, nchunks, 4], i16,
                            kind="ExternalInput").ap()
    basesw_d = nc.dram_tensor("basesw", [128, NB, 2, 2, 128], i16,
                              kind="ExternalInput").ap()
    bias_d = nc.dram_tensor("bias", [128, 2], f32, kind="ExternalInput").ap()
    out_d = nc.dram_tensor("out", [2, 128, dcore], i16,
                           kind="ExternalOutput").ap()

    # chunk index -> (slot, is_first_chunk_of_slot, is_last_chunk_of_slot)
    chunk_slot = []
    for s, cw in enumerate(slot_cws):
        for c in range(cw):
            chunk_slot.append((s, c == 0, c == cw - 1))

    with tile.TileContext(nc) as tc:
        with (
            tc.tile_pool(name="const", bufs=1) as cpool,
            tc.tile_pool(name="gh", bufs=3) as gpool,
            tc.tile_pool(name="w1h", bufs=2) as wpool,
            tc.tile_pool(name="oh", bufs=2) as ohpool,
            tc.tile_pool(name="stg", bufs=2) as apool,
            tc.tile_pool(name="ost", bufs=4) as opool,
            tc.tile_pool(name="ps1", bufs=2, space="PSUM") as ps1,
            tc.tile_pool(name="ps2", bufs=2, space="PSUM") as ps2,
        ):
            # ---- prologue: constants ----
            meta_sb = cpool.tile([128, nchunks, 4], i16)
            nc.sync.dma_start(meta_sb[:], meta_d[:])
            meta_bf = meta_sb[:].bitcast(bf16)
            basesw_sb = cpool.tile([128, NB, 2, 2, 128], i16)
            nc.sync.dma_start(basesw_sb[:], basesw_d[:])
            bias_sb = cpool.tile([128, 2], f32)
            nc.sync.dma_start(bias_sb[:], bias_d[:])
            iota_sb = cpool.tile([128, W], bf16)
            nc.gpsimd.iota(iota_sb[:], pattern=[[1, W]], base=0,
                           channel_multiplier=0,
                           allow_small_or_imprecise_dtypes=True)

            slab_tiles = {}

            def ensure_slab(si):
                """Issue the G-slab DMA + on-chip w1h build for slab si."""
                if si in slab_tiles or si >= nslabs:
                    return
                lo = si * TSLAB
                hi = min(nchunks, lo + TSLAB)
                t = hi - lo
                G = gpool.tile([128, TSLAB, F], i16, tag="G", name="G")
                nc.sync.dma_start(G[:, 0:t, :], gh_d[:, lo:hi, :])
                oh = ohpool.tile([128, TSLAB, W], bf16, tag="oh", name="oh")
                dstl = meta_bf[:, lo:hi, 0:1].broadcast_to([128, t, W])
                iota_b = iota_sb[:].unsqueeze(1).broadcast_to([128, t, W])
                nc.vector.tensor_tensor(oh[:, 0:t, :], iota_b, dstl,
                                        mybir.AluOpType.is_equal)
                w1h = wpool.tile([128, TSLAB, NB, W], bf16, tag="W", name="W")
                oh_b = oh[:, 0:t, :].unsqueeze(2).broadcast_to(
                    [128, t, NB, W])
                cv_b = meta_bf[:, lo:hi, 1:4].unsqueeze(3).broadcast_to(
                    [128, t, NB, W])
                nc.vector.tensor_tensor(w1h[:, 0:t, :, :], oh_b, cv_b,
                                        mybir.AluOpType.mult)
                slab_tiles[si] = (G, w1h)

            def emit_stage2(g):
                """Matmul group g's staged features against the bases."""
                stg = staging[g]
                for ohalf in range(2):
                    p2 = ps2.tile([128, GRP * W], f32, tag=f"p2o{ohalf}",
                                  name=f"p2o{ohalf}")
                    k = 0
                    for b in range(NB):
                        for h in range(2):
                            nc.tensor.matmul(
                                p2[:],
                                basesw_sb[:, b, h, ohalf, :].bitcast(bf16),
                                stg[:, h, b, :, :],
                                start=(k == 0), stop=(k == 2 * NB - 1))
                            k += 1
                    osb = opool.tile([128, GRP * W], bf16, tag=f"osb{ohalf}",
                                     name=f"osb{ohalf}")
                    nc.scalar.activation(
                        osb[:], p2[:], mybir.ActivationFunctionType.Relu,
                        bias=bias_sb[:, ohalf:ohalf + 1])
                    nc.sync.dma_start(
                        out_d[ohalf, :, g * GRP * W:(g + 1) * GRP * W],
                        osb[:].bitcast(i16))
                del staging[g]

            # ---- main pipeline over window pairs ----
            staging = {}
            cglob = 0
            ensure_slab(0)
            for j in range(npairs):
                p1 = [ps1.tile([128, PW, NB, W], f32, tag=f"p1h{h}",
                               name=f"p1h{h}") for h in range(2)]
                for w01 in range(PW):
                    s = PW * j + w01
                    for c in range(slot_cws[s]):
                        si, cloc = divmod(cglob, TSLAB)
                        ensure_slab(si)
                        ensure_slab(si + 1)
                        G, w1h = slab_tiles[si]
                        first = (c == 0)
                        last = (c == slot_cws[s] - 1)
                        for h in range(2):
                            nc.tensor.matmul(
                                p1[h][:, w01, :, :],
                                G[:, cloc, h * 128:(h + 1) * 128].bitcast(
                                    bf16),
                                w1h[:, cloc, :, :],
                                start=first, stop=last)
                        cglob += 1
                # defer stage2 of group (j-2)//2 to keep PE fed
                if j >= 2 and j % 2 == 0:
                    emit_stage2((j - 2) // 2)
                g = j // 2
                if g not in staging:
                    staging[g] = apool.tile([128, 2, NB, GRP, W], bf16,
                                            tag="stg", name="stg")
                for h in range(2):
                    a = PW * (j % 2)
                    dst_ap = staging[g][:, h, :, a:a + PW,
                                        :].transpose([0, 2, 1, 3])
                    # psum->staging casts ride the mostly-idle scalar engine
                    nc.scalar.copy(dst_ap, p1[h][:])
            for g in sorted(staging):
                emit_stage2(g)

    nc.compile()
    return nc


def host_prep(h_bf, src, dst, rel, comp, n_nodes, n_cores):
    """Sort/deal/pad edges; pre-gather h and build the metadata stream."""
    dcore = n_nodes // n_cores
    nw = dcore // W
    ngw = n_cores * nw
    gw = (dst // W).astype(np.int64)
    order = np.argsort(gw, kind="stable")
    counts = np.bincount(gw, minlength=ngw)
    starts = np.concatenate([[0], np.cumsum(counts)])

    # deal windows to cores by descending count; slot capacity = group max
    ranked = np.argsort(-counts, kind="stable")
    slot_cws = [max(1, -(-int(counts[ranked[n_cores * i]]) // 128))
                for i in range(nw)]
    nchunks = sum(slot_cws)
    epad = nchunks * 128

    w_edge = comp[rel].astype(ml_dtypes.bfloat16)        # [E, NB]
    dstloc = (dst % W).astype(np.float32).astype(ml_dtypes.bfloat16)

    gh = np.zeros((n_cores, 128, nchunks, F), np.int16)
    meta = np.zeros((n_cores, 128, nchunks, 4), ml_dtypes.bfloat16)
    win_of_slot = np.zeros((n_cores, nw), np.int64)

    slot_base = np.concatenate([[0], np.cumsum(slot_cws)])[:-1]
    srcs_flat = np.zeros((n_cores, epad), np.int64)
    meta_flat = np.zeros((n_cores, epad, 4), ml_dtypes.bfloat16)
    valid = np.zeros((n_cores, epad), bool)
    for k in range(n_cores):
        for i in range(nw):
            wid = int(ranked[n_cores * i + k])
            win_of_slot[k, i] = wid
            es = order[starts[wid]:starts[wid + 1]]
            base = slot_base[i] * 128
            n = len(es)
            srcs_flat[k, base:base + n] = src[es]
            valid[k, base:base + n] = True
            meta_flat[k, base:base + n, 0] = dstloc[es]
            meta_flat[k, base:base + n, 1:4] = w_edge[es]
    for k in range(n_cores):
        g = h_bf[srcs_flat[k]]                           # [epad, F] int16
        g[~valid[k]] = 0
        gh[k] = g.reshape(nchunks, 128, F).transpose(1, 0, 2)
        meta[k] = meta_flat[k].reshape(nchunks, 128, 4).transpose(1, 0, 2)
    return gh, meta, tuple(slot_cws), win_of_slot


def rgcn_kernel(text, src, dst, rel, bases, comp, bias, n_cores=8,
                run_fn=None, nc_cache={}):
    """Full-input kernel: shard, run on 8 cores, reassemble output."""
    Bt, St, INF = text.shape
    n_nodes = Bt * St
    h = text.reshape(n_nodes, INF)

    src = np.asarray(src).astype(np.int64)
    dst = np.asarray(dst).astype(np.int64)
    rel = np.asarray(rel).astype(np.int64)
    bases_np = np.asarray(bases, np.float32)
    comp_np = np.asarray(comp, np.float32)
    bias_np = np.asarray(bias, np.float32)

    h_bf = np.asarray(h, np.float32).astype(ml_dtypes.bfloat16).view(np.int16)
    gh, meta, slot_cws, win_of_slot = host_prep(
        h_bf, src, dst, rel, comp_np, n_nodes, n_cores)
    key = (n_nodes, slot_cws, n_cores)
    if key not in nc_cache:
        nc_cache[key] = build_program(n_nodes, slot_cws, n_cores)
    nc = nc_cache[key]

    # bases[b, f, o] -> basesw[p, b, h, oh, q] with f = h*128+p, o = oh*128+q
    bw = bases_np.astype(ml_dtypes.bfloat16).view(np.int16)
    basesw = np.ascontiguousarray(
        bw.reshape(NB, 2, 128, 2, 128).transpose(2, 0, 1, 3, 4))
    bias_w = np.ascontiguousarray(
        bias_np.reshape(2, 128).T.astype(np.float32))

    in_maps = [
        dict(gh=gh[k], meta=meta[k].view(np.int16), basesw=basesw,
             bias=bias_w)
        for k in range(n_cores)
    ]
    from concourse.bass_utils import run_bass_kernel_spmd
    if run_fn is None:
        res = run_bass_kernel_spmd(nc, in_maps, list(range(n_cores)))
        outs = [res.results[k]["out"] for k in range(n_cores)]
    else:
        outs = run_fn(nc, in_maps)

    out = np.zeros((n_nodes, O), np.float32)
    nw = len(slot_cws)
    for k in range(n_cores):
        ok = outs[k].view(ml_dtypes.bfloat16).astype(np.float32)
        ok = ok.reshape(O, nw, W)                        # [o, slot, d]
        for i in range(nw):
            wid = win_of_slot[k][i]
            out[wid * W:(wid + 1) * W] = ok[:, i, :].T
    return out.reshape(Bt, St, O)


_NC_CACHE = {}


def kernel(text, src, dst, rel, bases, comp, bias):
    out = rgcn_kernel(
        np.asarray(text, np.float32),
        np.asarray(src), np.asarray(dst), np.asarray(rel),
        np.asarray(bases, np.float32), np.asarray(comp, np.float32),
        np.asarray(bias, np.float32),
        n_cores=8, nc_cache=_NC_CACHE)
    return np.ascontiguousarray(out, np.float32)


# revision 20
# speedup vs baseline: 1.9652x; 1.0201x over previous
"""Self-contained TRN2 Bass kernel for the RGCN message-passing problem.

kernel(**inputs) takes the FULL unsharded inputs (text, src, dst, rel,
bases, comp, bias), shards edges by destination window across the 8
NeuronCores, runs the SPMD Bass program via run_bass_kernel_spmd, and
returns the full [64, 512, 256] float32 output.

Design (v2):
  - Edges are grouped by destination window (W=64 dst rows) and dealt
    to cores by descending window edge-count.  Since the edge indices
    are known on the host, the h[src] gather is done host-side: the
    device streams a pre-gathered [128, nchunks, 256] bf16 tensor with
    large sequential HWDGE DMAs (no SWDGE descriptor generation).
  - The per-edge scatter weights (comp[rel] placed at column
    b*W + dst%W) are built ON-CHIP from an 8-byte/edge metadata stream
    with two DVE ops per slab: onehot = is_equal(iota, dstloc) and
    w1h = onehot * compvals (broadcast APs).
  - Stage 1: per 128-edge chunk, 2 matmuls (G feature-half stationary,
    w1h moving) accumulate A[f, (w2,b,d)] for a PAIR of windows in one
    PSUM tile.
  - Stage 2: flipped — bases halves are the stationary operand (full
    128-wide PE), aggregated features stream as 256 dst columns per
    group of 4 windows.  Bias+ReLU fused in the scalar-engine
    activation; output written as [2, 128, dcore] (feature-major) and
    transposed back on the host.
"""

import numpy as np
import ml_dtypes

import concourse.bass as bass
import concourse.tile as tile
from concourse import bacc, mybir

F = 256       # in features
O = 256       # out features
NB = 3        # bases
W = 32        # dst rows per window
TSLAB = 32    # chunks per DMA slab (32 * 128 * 512B = 2 MiB)
GRP = 256 // W            # windows per stage-2 group (GRP*W = 256 dst cols)
PW = 384 // (NB * W)      # windows per stage-1 PSUM tile (384 f32 cols)


def build_program(n_nodes, slot_cws, n_cores=8):
    slot_cws = list(slot_cws)
    nw = len(slot_cws)
    assert nw % GRP == 0
    nchunks = sum(slot_cws)
    dcore = nw * W
    nslabs = -(-nchunks // TSLAB)
    npairs = nw // PW

    bf16 = mybir.dt.bfloat16
    f32 = mybir.dt.float32
    i16 = mybir.dt.int16

    # bf16 DRAM I/O breaks NEFF load under the PJRT path; all bf16 payloads
    # travel as int16 containers and are bitcast on-chip.
    nc = bacc.Bacc("TRN2", target_bir_lowering=False, debug=False,
                   num_devices=n_cores)
    gh_d = nc.dram_tensor("gh", [128, nchunks, F], i16,
                          kind="ExternalInput").ap()
    meta_d = nc.dram_tensor("meta", [128, nchunks, 4], i16,
                            kind="ExternalInput").ap()
    basesw_d = nc.dram_tensor("basesw", [128, NB, 2, 2, 128], i16,
                              kind="ExternalInput").ap()
    bias_d = nc.dram_tensor("bias", [128, 2], f32, kind="ExternalInput").ap()
    out_d = nc.dram_tensor("out", [2, 128, dcore], i16,
                           kind="ExternalOutput").ap()

    # chunk index -> (slot, is_first_chunk_of_slot, is_last_chunk_of_slot)
    chunk_slot = []
    for s, cw in enumerate(slot_cws):
        for c in range(cw):
            chunk_slot.append((s, c == 0, c == cw - 1))

    with tile.TileContext(nc) as tc:
        with (
            tc.tile_pool(name="const", bufs=1) as cpool,
            tc.tile_pool(name="gh", bufs=3) as gpool,
            tc.tile_pool(name="w1h", bufs=2) as wpool,
            tc.tile_pool(name="oh", bufs=2) as ohpool,
            tc.tile_pool(name="stg", bufs=2) as apool,
            tc.tile_pool(name="ost", bufs=4) as opool,
            tc.tile_pool(name="ps1", bufs=2, space="PSUM") as ps1,
            tc.tile_pool(name="ps2", bufs=2, space="PSUM") as ps2,
        ):
            # ---- prologue: constants ----
            # slab 0's metadata slice + G data go first so the PE can start
            # as early as possible; the rest of the constants follow.
            t0 = min(nchunks, TSLAB)
            meta_sb = cpool.tile([128, nchunks, 4], i16)
            nc.sync.dma_start(meta_sb[:, 0:t0, :], meta_d[:, 0:t0, :])
            meta_bf = meta_sb[:].bitcast(bf16)
            iota_sb = cpool.tile([128, W], bf16)
            nc.gpsimd.iota(iota_sb[:], pattern=[[1, W]], base=0,
                           channel_multiplier=0,
                           allow_small_or_imprecise_dtypes=True)

            slab_tiles = {}

            def ensure_slab(si):
                """Issue the G-slab DMA + on-chip w1h build for slab si."""
                if si in slab_tiles or si >= nslabs:
                    return
                lo = si * TSLAB
                hi = min(nchunks, lo + TSLAB)
                t = hi - lo
                G = gpool.tile([128, TSLAB, F], i16, tag="G", name="G")
                mid = (t + 1) // 2
                nc.sync.dma_start(G[:, 0:mid, :], gh_d[:, lo:lo + mid, :])
                nc.sync.dma_start(G[:, mid:t, :], gh_d[:, lo + mid:hi, :])
                oh = ohpool.tile([128, TSLAB, W], bf16, tag="oh", name="oh")
                dstl = meta_bf[:, lo:hi, 0:1].broadcast_to([128, t, W])
                iota_b = iota_sb[:].unsqueeze(1).broadcast_to([128, t, W])
                nc.vector.tensor_tensor(oh[:, 0:t, :], iota_b, dstl,
                                        mybir.AluOpType.is_equal)
                w1h = wpool.tile([128, TSLAB, NB, W], bf16, tag="W", name="W")
                oh_b = oh[:, 0:t, :].unsqueeze(2).broadcast_to(
                    [128, t, NB, W])
                cv_b = meta_bf[:, lo:hi, 1:4].unsqueeze(3).broadcast_to(
                    [128, t, NB, W])
                nc.vector.tensor_tensor(w1h[:, 0:t, :, :], oh_b, cv_b,
                                        mybir.AluOpType.mult)
                slab_tiles[si] = (G, w1h)

            def emit_stage2(g):
                """Matmul group g's staged features against the bases."""
                stg = staging[g]
                for ohalf in range(2):
                    p2 = ps2.tile([128, GRP * W], f32, tag=f"p2o{ohalf}",
                                  name=f"p2o{ohalf}")
                    k = 0
                    for b in range(NB):
                        for h in range(2):
                            nc.tensor.matmul(
                                p2[:],
                                basesw_sb[:, b, h, ohalf, :].bitcast(bf16),
                                stg[:, h, b, :, :],
                                start=(k == 0), stop=(k == 2 * NB - 1))
                            k += 1
                    osb = opool.tile([128, GRP * W], bf16, tag=f"osb{ohalf}",
                                     name=f"osb{ohalf}")
                    nc.scalar.activation(
                        osb[:], p2[:], mybir.ActivationFunctionType.Relu,
                        bias=bias_sb[:, ohalf:ohalf + 1])
                    # scalar-engine DGE queue: don't sit behind 2MB G slabs
                    nc.scalar.dma_start(
                        out_d[ohalf, :, g * GRP * W:(g + 1) * GRP * W],
                        osb[:].bitcast(i16))
                del staging[g]

            # ---- main pipeline over window pairs ----
            staging = {}
            cglob = 0
            ensure_slab(0)
            basesw_sb = cpool.tile([128, NB, 2, 2, 128], i16)
            nc.sync.dma_start(basesw_sb[:], basesw_d[:])
            bias_sb = cpool.tile([128, 2], f32)
            nc.sync.dma_start(bias_sb[:], bias_d[:])
            if nchunks > t0:
                nc.sync.dma_start(meta_sb[:, t0:, :], meta_d[:, t0:, :])
            ensure_slab(1)
            for j in range(npairs):
                p1 = [ps1.tile([128, PW, NB, W], f32, tag=f"p1h{h}",
                               name=f"p1h{h}") for h in range(2)]
                for w01 in range(PW):
                    s = PW * j + w01
                    for c in range(slot_cws[s]):
                        si, cloc = divmod(cglob, TSLAB)
                        ensure_slab(si)
                        ensure_slab(si + 1)
                        G, w1h = slab_tiles[si]
                        first = (c == 0)
                        last = (c == slot_cws[s] - 1)
                        for h in range(2):
                            nc.tensor.matmul(
                                p1[h][:, w01, :, :],
                                G[:, cloc, h * 128:(h + 1) * 128].bitcast(
                                    bf16),
                                w1h[:, cloc, :, :],
                                start=first, stop=last)
                        cglob += 1
                # defer stage2 of group (j-2)//2 to keep PE fed
                if j >= 2 and j % 2 == 0:
                    emit_stage2((j - 2) // 2)
                g = j // 2
                if g not in staging:
                    staging[g] = apool.tile([128, 2, NB, GRP, W], bf16,
                                            tag="stg", name="stg")
                for h in range(2):
                    a = PW * (j % 2)
                    dst_ap = staging[g][:, h, :, a:a + PW,
                                        :].transpose([0, 2, 1, 3])
                    # psum->staging casts ride the mostly-idle scalar engine
                    nc.scalar.copy(dst_ap, p1[h][:])
            for g in sorted(staging):
                emit_stage2(g)

    nc.compile()
    return nc


def host_prep(h_bf, src, dst, rel, comp, n_nodes, n_cores):
    """Sort/deal/pad edges; pre-gather h and build the metadata stream."""
    dcore = n_nodes // n_cores
    nw = dcore // W
    ngw = n_cores * nw
    gw = (dst // W).astype(np.int64)
    order = np.argsort(gw, kind="stable")
    counts = np.bincount(gw, minlength=ngw)
    starts = np.concatenate([[0], np.cumsum(counts)])

    # deal windows to cores by descending count; slot capacity = group max
    ranked = np.argsort(-counts, kind="stable")
    slot_cws = [max(1, -(-int(counts[ranked[n_cores * i]]) // 128))
                for i in range(nw)]
    nchunks = sum(slot_cws)
    epad = nchunks * 128

    w_edge = comp[rel].astype(ml_dtypes.bfloat16)        # [E, NB]
    dstloc = (dst % W).astype(np.float32).astype(ml_dtypes.bfloat16)

    gh = np.zeros((n_cores, 128, nchunks, F), np.int16)
    meta = np.zeros((n_cores, 128, nchunks, 4), ml_dtypes.bfloat16)
    win_of_slot = np.zeros((n_cores, nw), np.int64)

    slot_base = np.concatenate([[0], np.cumsum(slot_cws)])[:-1]
    srcs_flat = np.zeros((n_cores, epad), np.int64)
    meta_flat = np.zeros((n_cores, epad, 4), ml_dtypes.bfloat16)
    valid = np.zeros((n_cores, epad), bool)
    for k in range(n_cores):
        for i in range(nw):
            wid = int(ranked[n_cores * i + k])
            win_of_slot[k, i] = wid
            es = order[starts[wid]:starts[wid + 1]]
            base = slot_base[i] * 128
            n = len(es)
            srcs_flat[k, base:base + n] = src[es]
            valid[k, base:base + n] = True
            meta_flat[k, base:base + n, 0] = dstloc[es]
            meta_flat[k, base:base + n, 1:4] = w_edge[es]
    for k in range(n_cores):
        g = h_bf[srcs_flat[k]]                           # [epad, F] int16
        g[~valid[k]] = 0
        gh[k] = g.reshape(nchunks, 128, F).transpose(1, 0, 2)
        meta[k] = meta_flat[k].reshape(nchunks, 128, 4).transpose(1, 0, 2)
    return gh, meta, tuple(slot_cws), win_of_slot


def rgcn_kernel(text, src, dst, rel, bases, comp, bias, n_cores=8,
                run_fn=None, nc_cache={}):
    """Full-input kernel: shard, run on 8 cores, reassemble output."""
    Bt, St, INF = text.shape
    n_nodes = Bt * St
    h = text.reshape(n_nodes, INF)

    src = np.asarray(src).astype(np.int64)
    dst = np.asarray(dst).astype(np.int64)
    rel = np.asarray(rel).astype(np.int64)
    bases_np = np.asarray(bases, np.float32)
    comp_np = np.asarray(comp, np.float32)
    bias_np = np.asarray(bias, np.float32)

    h_bf = np.asarray(h, np.float32).astype(ml_dtypes.bfloat16).view(np.int16)
    gh, meta, slot_cws, win_of_slot = host_prep(
        h_bf, src, dst, rel, comp_np, n_nodes, n_cores)
    key = (n_nodes, slot_cws, n_cores)
    if key not in nc_cache:
        nc_cache[key] = build_program(n_nodes, slot_cws, n_cores)
    nc = nc_cache[key]

    # bases[b, f, o] -> basesw[p, b, h, oh, q] with f = h*128+p, o = oh*128+q
    bw = bases_np.astype(ml_dtypes.bfloat16).view(np.int16)
    basesw = np.ascontiguousarray(
        bw.reshape(NB, 2, 128, 2, 128).transpose(2, 0, 1, 3, 4))
    bias_w = np.ascontiguousarray(
        bias_np.reshape(2, 128).T.astype(np.float32))

    in_maps = [
        dict(gh=gh[k], meta=meta[k].view(np.int16), basesw=basesw,
             bias=bias_w)
        for k in range(n_cores)
    ]
    from concourse.bass_utils import run_bass_kernel_spmd
    if run_fn is None:
        res = run_bass_kernel_spmd(nc, in_maps, list(range(n_cores)))
        outs = [res.results[k]["out"] for k in range(n_cores)]
    else:
        outs = run_fn(nc, in_maps)

    out = np.zeros((n_nodes, O), np.float32)
    nw = len(slot_cws)
    for k in range(n_cores):
        ok = outs[k].view(ml_dtypes.bfloat16).astype(np.float32)
        ok = ok.reshape(O, nw, W)                        # [o, slot, d]
        for i in range(nw):
            wid = win_of_slot[k][i]
            out[wid * W:(wid + 1) * W] = ok[:, i, :].T
    return out.reshape(Bt, St, O)


_NC_CACHE = {}


def kernel(text, src, dst, rel, bases, comp, bias):
    out = rgcn_kernel(
        np.asarray(text, np.float32),
        np.asarray(src), np.asarray(dst), np.asarray(rel),
        np.asarray(bases, np.float32), np.asarray(comp, np.float32),
        np.asarray(bias, np.float32),
        n_cores=8, nc_cache=_NC_CACHE)
    return np.ascontiguousarray(out, np.float32)
